# revision 25
# baseline (speedup 1.0000x reference)
"""Contextual attention kernel for Trainium2 (8 NeuronCores, data-parallel over batch).

Math (per batch b):
    Q = feaQK @ q_w.T + q_b
    k3 = conv1d(feaQK.T, cn3_w, SAME) + b3 ; k5 = conv1d(..., cn5_w) + b5
    K = [feaQK, k3, k5] @ k_w.T + k_b
    V = feaV @ v_w.T + v_b
    S = (Q @ K.T) / sqrt(D); mask keys >= seqlen with -inf
    out = softmax(S) @ V + V

Kernel strategy:
  * The convs + concat + K-projection collapse into a single width-5 stencil:
        K[s] = sum_{d=-2..2} feaQK[s+d] @ Wk[d] + kb_eff
    with Wk composed on the host (15 matmul-units of work -> 9).
  * All activations live on-chip in transposed layout ([feature, seq]) so no
    on-device transposes are needed anywhere:
        QT/KT from xT (host-transposed feaQK, zero-padded cols)
        scoresT[k,q] = KT chunks (stationary) x QT  (PSUM fp32)
        ET16 = 16*exp(scoresT/32 + mask)  (mask folded into exp bias)
        V0 (natural [s,d]) from host-transposed feaV as the stationary operand
        outU[q,d] = ET16 chunks (stationary) x V8; den[q] = ET16 x ones
        out = outU / den + (V0 + 2*vb)        [atten@vb == vb since sum(atten)=1]
  * fp8e4 DoubleRow matmuls (2 contraction blocks / instruction, ~1.5-1.8x bf16)
    for the Q-proj, K-stencil, scores and PV stages. Weights are scaled x256 on
    the host so they sit in fp8's normal range; Q/K are stored x16; ET x16.
    The V-projection stays bf16: out ~= V, so V's accuracy dominates the
    output and fp8 would blow the error budget. The PV matmul uses a separate
    fp8 copy (V8) of the unbiased projection.
  * Keys beyond seqlength are dead: K/scores/PV work only covers the first
    ceil(seqlen/128) key chunks per batch slot. Batches are paired
    longest-with-shortest across cores so the compile-time per-slot chunk
    counts (max over cores) stay small; sub-chunk masking still goes through
    the exp bias, so over-covering is always correct.
  * 16 batches -> 2 per core, full weights on every core. Output DMA in bf16.
"""

import numpy as np
import ml_dtypes

from concourse import bacc
import concourse.tile as tile
from concourse import mybir

B, S, C, D = 16, 1024, 1024, 1024
P = 128
NCI, NDI, NKI, NQI, NSI = C // P, D // P, S // P, S // P, S // P
NF = 512  # matmul free dim (one PSUM bank of fp32)
PADL = 2   # left zero pad for the width-5 stencil
SP = 1040  # padded seq width; multiple of 16 so fp8 DoubleRow pair-stride is legal
LB = 2  # local batches per core
NCORES = 8
MASK_NEG = -60000.0
SCALE = 1.0 / 32.0  # 1/sqrt(D)
WS = 256.0   # host weight scale into fp8 normal range
AS = 16.0    # on-chip activation scale for QT/KT/ET
LOG_AS = float(np.log(AS))

BF = mybir.dt.bfloat16
F32 = mybir.dt.float32
F8 = mybir.dt.float8e4
AF = mybir.ActivationFunctionType
DR = mybir.MatmulPerfMode.DoubleRow

TRACE = False  # set by test harness to collect HW profile
_CACHE = {}
MARKS = []  # (label, first-instruction-name) per stage, for trace attribution


def _build_program(vs, ls):
    nc = bacc.Bacc("TRN2", dynamic_dma_scratch_size=256)

    xt = nc.dram_tensor("xt", [LB, NCI // 2, P, 2, SP], F8, kind="ExternalInput")
    fvt = nc.dram_tensor("fvt", [LB, NCI // 2, P, 2, S], BF, kind="ExternalInput")
    wq = nc.dram_tensor("wq", [NDI, P, NCI, P], F8, kind="ExternalInput")
    wk = nc.dram_tensor("wk", [5, P, NCI, D], F8, kind="ExternalInput")
    wv = nc.dram_tensor("wv", [NCI // 2, P, 2, D], BF, kind="ExternalInput")
    qb = nc.dram_tensor("qb", [P, NDI], F32, kind="ExternalInput")
    kb = nc.dram_tensor("kb", [P, NDI], F32, kind="ExternalInput")
    vb = nc.dram_tensor("vb", [P, D], F32, kind="ExternalInput")
    mb = nc.dram_tensor("mb", [LB, P, NKI], F32, kind="ExternalInput")
    out = nc.dram_tensor("out", [LB, S, D], BF, kind="ExternalOutput")

    with tile.TileContext(nc) as tc:
        _emit(nc, tc, xt, fvt, wq, wk, wv, qb, kb, vb, mb, out, vs, ls)
    nc.finalize()
    return nc


def _mark(nc, label):
    mx = 0
    for k in nc._state.inst_map:
        if k.startswith("I-"):
            try:
                mx = max(mx, int(k[2:].split("_")[0]))
            except ValueError:
                pass
    MARKS.append((label, mx))


def _emit(nc, tc, xt, fvt, wq, wk, wv, qb, kb, vb, mb, out, vs, ls):
    from contextlib import ExitStack

    with ExitStack() as ctx:
        wpool = ctx.enter_context(tc.tile_pool(name="wpool", bufs=1))
        apool = ctx.enter_context(tc.tile_pool(name="apool", bufs=1))
        opool = ctx.enter_context(tc.tile_pool(name="opool", bufs=3))
        spool = ctx.enter_context(tc.tile_pool(name="spool", bufs=2))
        pp = ctx.enter_context(tc.tile_pool(name="pp", bufs=6, space="PSUM"))
        pd = ctx.enter_context(tc.tile_pool(name="pd", bufs=2, space="PSUM"))

        # PE warm-up: ~10 dependency-free matmuls on junk SBUF so the HAM
        # clock gate reaches 8/8 while the first input DMAs are in flight.
        ONES = wpool.tile([P, 2, 16], F8, tag="ones")
        nc.vector.memset(ONES, 1.0)
        JW = wpool.tile([P, 2, P], F8, tag="jw")
        nc.vector.memset(JW, 1.0)
        JM = wpool.tile([P, 2, NF], F8, tag="jm")
        nc.vector.memset(JM, 0.0)
        for w in range(6):
            wps = pp.tile([P, NF], F32, tag="ps", name="warm")
            nc.tensor.matmul(wps, JW, JM, start=True, stop=True, perf_mode=DR)
        QB = wpool.tile([P, NDI], F32, tag="qb")
        KB = wpool.tile([P, NDI], F32, tag="kb")
        VB2 = wpool.tile([P, D], F32, tag="vb")
        WV = wpool.tile([P, NCI, D], BF, tag="wv")
        WQ = wpool.tile([P, NCI, D], F8, tag="wq")
        WK = None

        for b in range(LB):
            v = vs[b]   # valid key chunks for this batch slot
            L = ls[b]   # exact covered key width (<= v*128, multiple of 16)
            # key-dim psum groups: (offset, width) pieces covering L cols.
            # Balanced widths keep every matmul stream-bound; a (512, small)
            # split leaves the small group at the ~60ns NX dispatch floor.
            if L <= NF:
                kg = [(0, L)]
            else:
                w0 = ((L + 1) // 2 + 15) // 16 * 16
                kg = [(0, w0), (w0, L - w0)]

            # --- stage B: QT[d, s] = 16*Q (fp8 DoubleRow; smallest DMA lead-in)
            XT = apool.tile([P, NCI, SP], F8, tag="xt")
            # DMA issue ops cost ~600ns each regardless of size, so batch two
            # 128-chunk loads per XT issue.  WQ instead loads per di column
            # slice: B's psum group di only reads WQ[:, :, di*128:...], so
            # group 0 needs 1.2MB (XT + slice 0) instead of 2.1MB before it
            # can finish — early DMA bandwidth couldn't deliver the latter in
            # time.  b=0 interleaves both issue queues (ScalarE is free until
            # the first QT activation).
            def _xt(cp, eng):
                eng.dma_start(out=XT[:, 2 * cp:2 * cp + 2, :], in_=xt[b, cp])

            def _wq(di, eng):
                eng.dma_start(out=WQ[:, :, di * P:(di + 1) * P], in_=wq[di])

            if b == 0:
                _xt(0, nc.sync)
                _wq(0, nc.scalar)
                _wq(1, nc.sync)
                _xt(1, nc.scalar)
                _xt(2, nc.sync)
                _wq(2, nc.scalar)
                _wq(3, nc.sync)
                _xt(3, nc.scalar)
                for di in range(4, NDI):
                    _wq(di, nc.scalar if di % 2 == 0 else nc.sync)
            else:
                for cp in range(NCI // 2):
                    _xt(cp, nc.sync)
            MB = spool.tile([P, NKI], F32, tag="mb")
            nc.sync.dma_start(out=MB, in_=mb[b])
            if b == 0:
                nc.sync.dma_start(out=QB, in_=qb[:, :])
                nc.sync.dma_start(out=KB, in_=kb[:, :])
                nc.sync.dma_start(out=VB2, in_=vb[:, :])
            _mark(nc, f"B{b}")
            QT = apool.tile([P, NDI, S], F8, tag="qt")
            for di in range(NDI):
                ps = [pp.tile([P, NF], F32, tag="ps", name=f"ps{_i}") for _i in range(2)]
                for cp in range(0, NCI, 2):
                    lhsT = WQ[:, cp:cp + 2, di * P:(di + 1) * P]
                    for sh in range(2):
                        nc.tensor.matmul(
                            ps[sh], lhsT,
                            XT[:, cp:cp + 2, PADL + sh * NF: PADL + sh * NF + NF],
                            start=(cp == 0), stop=(cp == NCI - 2), perf_mode=DR)
                for sh in range(2):
                    nc.scalar.activation(
                        QT[:, di, sh * NF:(sh + 1) * NF], ps[sh], AF.Identity,
                        bias=QB[:, di:di + 1], scale=AS / WS)

            # --- stage D: V0 natural [s, d]; bf16 Vb2 for +V, fp8 V8 for PV --
            FVT = apool.tile([P, NCI, S], BF, tag="fvt")
            for cp in range(NCI // 2):
                nc.sync.dma_start(out=FVT[:, 2 * cp:2 * cp + 2, :], in_=fvt[b, cp])
                if b == 0:
                    nc.sync.dma_start(out=WV[:, 2 * cp:2 * cp + 2, :], in_=wv[cp])
            _mark(nc, f"D{b}")
            V = apool.tile([P, NSI, D], BF, tag="v")
            V8 = apool.tile([P, NSI, D], F8, tag="v8")
            for si in range(NSI):
                ps = [pp.tile([P, NF], F32, tag="ps", name=f"ps{_i}") for _i in range(2)]
                for ci in range(NCI):
                    lhsT = FVT[:, ci, si * P:(si + 1) * P]
                    for dh in range(2):
                        nc.tensor.matmul(
                            ps[dh], lhsT, WV[:, ci, dh * NF:(dh + 1) * NF],
                            start=(ci == 0), stop=(ci == NCI - 1))
                for dh in range(2):
                    nc.vector.tensor_add(
                        V[:, si, dh * NF:(dh + 1) * NF], ps[dh],
                        VB2[:, dh * NF:(dh + 1) * NF])
                    if si < v:
                        # ScalarE is idle during stage D; without this the DVE
                        # runs ~2.4us/group against the PE's 1.76us cadence
                        nc.scalar.activation(
                            V8[:, si, dh * NF:(dh + 1) * NF], ps[dh],
                            AF.Copy, bias=0.0, scale=1.0)

            # --- stage C: KT[d, s] = 16*K (width-5 stencil, only v key chunks)
            if WK is None:
                WK = []
                for j in range(5):
                    t = wpool.tile([P, NCI, D], F8, tag=f"wk{j}")
                    nc.sync.dma_start(out=t, in_=wk[j])
                    WK.append(t)
            _mark(nc, f"C{b}")
            KT = apool.tile([P, NDI, S], F8, tag="kt")
            if L < v * P:
                # stage E reads whole 128-col chunks; zero the K columns the
                # stencil no longer computes (they are all masked anyway)
                nc.vector.memset(KT[:, :, L:v * P], 0.0)
            for di in range(NDI):
                ps = [pp.tile([P, NF], F32, tag="ps", name=f"ps{_i}")
                      for _i in range(len(kg))]
                step, nsteps = 0, 5 * (NCI // 2)
                for j in range(5):
                    for cp in range(0, NCI, 2):
                        lhsT = WK[j][:, cp:cp + 2, di * P:(di + 1) * P]
                        for g, (off, w) in enumerate(kg):
                            nc.tensor.matmul(
                                ps[g][:, :w], lhsT,
                                XT[:, cp:cp + 2, j + off: j + off + w],
                                start=(step == 0), stop=(step == nsteps - 1),
                                perf_mode=DR)
                        step += 1
                for g, (off, w) in enumerate(kg):
                    nc.scalar.activation(
                        KT[:, di, off:off + w], ps[g][:, :w], AF.Identity,
                        bias=KB[:, di:di + 1], scale=AS / WS)

            # --- stage E: ET16[k, q] = 16*exp(scoresT/32 + mask) -------------
            _mark(nc, f"E{b}")
            ET = apool.tile([P, NKI, S], F8, tag="et")
            for ki in range(v):
                ps = [pp.tile([P, NF], F32, tag="ps", name=f"ps{_i}") for _i in range(2)]
                for dp in range(0, NDI, 2):
                    lhsT = KT[:, dp:dp + 2, ki * P:(ki + 1) * P]
                    for qh in range(2):
                        nc.tensor.matmul(
                            ps[qh], lhsT, QT[:, dp:dp + 2, qh * NF:(qh + 1) * NF],
                            start=(dp == 0), stop=(dp == NDI - 2), perf_mode=DR)
                for qh in range(2):
                    nc.scalar.activation(
                        ET[:, ki, qh * NF:(qh + 1) * NF], ps[qh], AF.Exp,
                        bias=MB[:, ki:ki + 1], scale=SCALE / (AS * AS))

            # --- stage F: out = (ET16^T @ V8) / den + Vb2 --------------------
            _mark(nc, f"F{b}")
            for qi in range(NQI):
                pso = [pp.tile([P, NF], F32, tag="ps", name=f"pso{_i}") for _i in range(2)]
                psd = pd.tile([P, 1], F32, tag="den")
                for kp in range(0, v - (v % 2), 2):
                    lhsT = ET[:, kp:kp + 2, qi * P:(qi + 1) * P]
                    st, sp_ = (kp == 0), (kp + 2 >= v)
                    for dh in range(2):
                        nc.tensor.matmul(
                            pso[dh], lhsT, V8[:, kp:kp + 2, dh * NF:(dh + 1) * NF],
                            start=st, stop=sp_, perf_mode=DR)
                    nc.tensor.matmul(psd, lhsT, ONES[:, :, 0:1],
                                     start=st, stop=sp_, perf_mode=DR)
                if v % 2:
                    ki = v - 1
                    lhsT = ET[:, ki, qi * P:(qi + 1) * P]
                    st = (v == 1)
                    for dh in range(2):
                        nc.tensor.matmul(
                            pso[dh], lhsT, V8[:, ki, dh * NF:(dh + 1) * NF],
                            start=st, stop=True)
                    nc.tensor.matmul(psd, lhsT, ONES[:, 0, 0:1],
                                     start=st, stop=True)
                REC = spool.tile([P, 1], F32, tag="rec")
                nc.vector.reciprocal(REC, psd)
                for dh in range(2):
                    OB = opool.tile([P, NF], BF, tag="obf", name=f"ob{dh}")
                    last = (b == LB - 1 and qi == NQI - 1)
                    if qi >= 2 and not (last and dh == 1):
                        # scalar does the x(1/den) move to bf16; the DVE add is
                        # then all-16-bit (2x rate) -> DVE drops from 1.67us to
                        # ~0.9us per group, ending den-matmul stalls on PSUM
                        OT = opool.tile([P, NF], BF, tag="otb", name=f"otb{dh}")
                        nc.scalar.activation(OT, pso[dh], AF.Copy,
                                             bias=0.0, scale=REC)
                        nc.vector.tensor_add(
                            OB, OT, V[:, qi, dh * NF:(dh + 1) * NF])
                    else:
                        nc.vector.scalar_tensor_tensor(
                            OB, pso[dh], REC, V[:, qi, dh * NF:(dh + 1) * NF],
                            mybir.AluOpType.mult, mybir.AluOpType.add)
                    eng = nc.sync if dh == 0 else nc.scalar
                    eng.dma_start(
                        out=out[b, qi * P:(qi + 1) * P, dh * NF:(dh + 1) * NF],
                        in_=OB)


def _prep_host(feaQK, feaV, seqlengths, cn3_w, cn3_b, cn5_w, cn5_b,
               k_w, k_b, q_w, q_b, v_w, v_b):
    """Compose weights, assign batches to cores, lay out per-core inputs."""
    f32 = np.float32
    bf16 = ml_dtypes.bfloat16
    f8 = ml_dtypes.float8_e4m3
    feaQK = np.asarray(feaQK, f32)
    feaV = np.asarray(feaV, f32)
    seqlengths = np.asarray(seqlengths).astype(np.int64)

    W1 = np.asarray(k_w, f32)[:, :C]
    W2 = np.asarray(k_w, f32)[:, C:2 * C]
    W3 = np.asarray(k_w, f32)[:, 2 * C:]

    wk = np.zeros((5, C, D), f32)  # [tap j (= shift+2), c, d]
    for t in range(3):
        wk[t + 1] += (W2 @ np.asarray(cn3_w, f32)[:, :, t]).T
    for t in range(5):
        wk[t] += (W3 @ np.asarray(cn5_w, f32)[:, :, t]).T
    wk[2] += W1.T
    kb_eff = (np.asarray(k_b, f32) + W2 @ np.asarray(cn3_b, f32)
              + W3 @ np.asarray(cn5_b, f32))

    wq = np.ascontiguousarray(np.asarray(q_w, f32).T)
    wv = np.ascontiguousarray(np.asarray(v_w, f32).T)

    qb_pd = np.ascontiguousarray((np.asarray(q_b, f32) * AS).reshape(NDI, P).T)
    kb_pd = np.ascontiguousarray((kb_eff * AS).reshape(NDI, P).T)
    vb2_rep = np.ascontiguousarray(
        np.broadcast_to(2.0 * np.asarray(v_b, f32), (P, D)))

    key_valid = np.arange(S)[None, :] < seqlengths[:, None]
    mask = np.where(key_valid, LOG_AS, MASK_NEG).astype(f32)  # [B, S]

    # Pair longest with shortest so the compile-time per-slot chunk counts
    # (max over cores) stay near the per-core optimum.
    vchunks = np.clip(np.ceil(seqlengths / P).astype(int), 1, NKI)
    order = np.argsort(-seqlengths, kind="stable")
    batch_of = np.zeros((NCORES, LB), int)
    for i in range(NCORES):
        batch_of[i, 0] = order[B - 1 - i]
        batch_of[i, 1] = order[i]
    vs = (int(vchunks[batch_of[:, 0]].max()),
          int(vchunks[batch_of[:, 1]].max()))
    ls = tuple(min(S, (int(seqlengths[batch_of[:, sl]].max()) + 15) // 16 * 16)
               for sl in range(LB))

    def pairs(a):  # [C, X] -> [NCI//2, P, 2, X] so one DMA covers 2 chunks
        return np.ascontiguousarray(
            a.reshape(NCI // 2, 2, P, -1).transpose(0, 2, 1, 3))

    wq_8 = np.ascontiguousarray(
        np.clip(wq * WS, -240, 240).astype(f8)
        .reshape(NCI, P, NDI, P).transpose(2, 1, 0, 3))
    wk_8 = np.ascontiguousarray(
        np.clip(wk * WS, -240, 240).astype(f8).reshape(5, NCI, P, D)
        .transpose(0, 2, 1, 3))
    wv_b = pairs(wv.astype(bf16))

    in_maps = []
    for core in range(NCORES):
        bs = batch_of[core]
        xts = np.zeros((LB, C, SP), f8)
        xts[:, :, PADL:PADL + S] = np.clip(
            feaQK[bs].transpose(0, 2, 1), -240, 240).astype(f8)
        xts = np.ascontiguousarray(
            xts.reshape(LB, NCI // 2, 2, P, SP).transpose(0, 1, 3, 2, 4))
        fvts = np.ascontiguousarray(
            feaV[bs].transpose(0, 2, 1).astype(bf16)
            .reshape(LB, NCI // 2, 2, P, S).transpose(0, 1, 3, 2, 4))
        mbs = np.ascontiguousarray(
            mask[bs].reshape(LB, NKI, P).transpose(0, 2, 1))
        in_maps.append({
            "xt": xts, "fvt": fvts,
            "wq": wq_8, "wk": wk_8, "wv": wv_b,
            "qb": qb_pd, "kb": kb_pd, "vb": vb2_rep, "mb": mbs,
        })
    return in_maps, batch_of, vs, ls


def kernel(**inputs):
    from concourse.bass_utils import run_bass_kernel_spmd

    in_maps, batch_of, vs, ls = _prep_host(**inputs)
    if _CACHE.get("vs") != (vs, ls):
        _CACHE["nc"] = _build_program(vs, ls)
        _CACHE["vs"] = (vs, ls)
    nc = _CACHE["nc"]
    res = run_bass_kernel_spmd(nc, in_maps, core_ids=list(range(NCORES)),
                               trace=TRACE)
    _CACHE["last_result"] = res
    full = np.zeros((B, S, D), np.float32)
    for core in range(NCORES):
        full[batch_of[core]] = res.results[core]["out"].astype(np.float32)
    return full


# revision 26
# speedup vs baseline: 1.0099x; 1.0099x over previous
"""Contextual attention kernel for Trainium2 (8 NeuronCores, data-parallel over batch).

Math (per batch b):
    Q = feaQK @ q_w.T + q_b
    k3 = conv1d(feaQK.T, cn3_w, SAME) + b3 ; k5 = conv1d(..., cn5_w) + b5
    K = [feaQK, k3, k5] @ k_w.T + k_b
    V = feaV @ v_w.T + v_b
    S = (Q @ K.T) / sqrt(D); mask keys >= seqlen with -inf
    out = softmax(S) @ V + V

Kernel strategy:
  * The convs + concat + K-projection collapse into a single width-5 stencil:
        K[s] = sum_{d=-2..2} feaQK[s+d] @ Wk[d] + kb_eff
    with Wk composed on the host (15 matmul-units of work -> 9).
  * All activations live on-chip in transposed layout ([feature, seq]) so no
    on-device transposes are needed anywhere:
        QT/KT from xT (host-transposed feaQK, zero-padded cols)
        scoresT[k,q] = KT chunks (stationary) x QT  (PSUM fp32)
        ET16 = 16*exp(scoresT/32 + mask)  (mask folded into exp bias)
        V0 (natural [s,d]) from host-transposed feaV as the stationary operand
        outU[q,d] = ET16 chunks (stationary) x V8; den[q] = ET16 x ones
        out = outU / den + (V0 + 2*vb)        [atten@vb == vb since sum(atten)=1]
  * fp8e4 DoubleRow matmuls (2 contraction blocks / instruction, ~1.5-1.8x bf16)
    for the Q-proj, K-stencil, scores and PV stages. Weights are scaled x256 on
    the host so they sit in fp8's normal range; Q/K are stored x16; ET x16.
    The V-projection stays bf16: out ~= V, so V's accuracy dominates the
    output and fp8 would blow the error budget. The PV matmul uses a separate
    fp8 copy (V8) of the unbiased projection.
  * Keys beyond seqlength are dead: K/scores/PV work only covers the first
    ceil(seqlen/128) key chunks per batch slot. Batches are paired
    longest-with-shortest across cores so the compile-time per-slot chunk
    counts (max over cores) stay small; sub-chunk masking still goes through
    the exp bias, so over-covering is always correct.
  * 16 batches -> 2 per core, full weights on every core. Output DMA in bf16.
"""

import numpy as np
import ml_dtypes

from concourse import bacc
import concourse.tile as tile
from concourse import mybir

B, S, C, D = 16, 1024, 1024, 1024
P = 128
NCI, NDI, NKI, NQI, NSI = C // P, D // P, S // P, S // P, S // P
NF = 512  # matmul free dim (one PSUM bank of fp32)
PADL = 2   # left zero pad for the width-5 stencil
SP = 1040  # padded seq width; multiple of 16 so fp8 DoubleRow pair-stride is legal
LB = 2  # local batches per core
NCORES = 8
MASK_NEG = -60000.0
SCALE = 1.0 / 32.0  # 1/sqrt(D)
WS = 256.0   # host weight scale into fp8 normal range
AS = 16.0    # on-chip activation scale for QT/KT/ET
LOG_AS = float(np.log(AS))

BF = mybir.dt.bfloat16
F32 = mybir.dt.float32
F8 = mybir.dt.float8e4
AF = mybir.ActivationFunctionType
DR = mybir.MatmulPerfMode.DoubleRow

TRACE = False  # set by test harness to collect HW profile
_CACHE = {}
MARKS = []  # (label, first-instruction-name) per stage, for trace attribution


def _build_program(vs, ls):
    nc = bacc.Bacc("TRN2", dynamic_dma_scratch_size=256)

    xt = nc.dram_tensor("xt", [LB, NCI // 2, P, 2, SP], F8, kind="ExternalInput")
    fvt = nc.dram_tensor("fvt", [LB, NCI // 2, P, 2, S], BF, kind="ExternalInput")
    wq = nc.dram_tensor("wq", [NCI // 2, P, 2, D], F8, kind="ExternalInput")
    wk = nc.dram_tensor("wk", [5, P, NCI, D], F8, kind="ExternalInput")
    wv = nc.dram_tensor("wv", [NCI // 2, P, 2, D], BF, kind="ExternalInput")
    qb = nc.dram_tensor("qb", [P, NDI], F32, kind="ExternalInput")
    kb = nc.dram_tensor("kb", [P, NDI], F32, kind="ExternalInput")
    vb = nc.dram_tensor("vb", [P, D], F32, kind="ExternalInput")
    mb = nc.dram_tensor("mb", [LB, P, NKI], F32, kind="ExternalInput")
    out = nc.dram_tensor("out", [LB, S, D], BF, kind="ExternalOutput")

    with tile.TileContext(nc) as tc:
        _emit(nc, tc, xt, fvt, wq, wk, wv, qb, kb, vb, mb, out, vs, ls)
    nc.finalize()
    return nc


def _mark(nc, label):
    mx = 0
    for k in nc._state.inst_map:
        if k.startswith("I-"):
            try:
                mx = max(mx, int(k[2:].split("_")[0]))
            except ValueError:
                pass
    MARKS.append((label, mx))


def _emit(nc, tc, xt, fvt, wq, wk, wv, qb, kb, vb, mb, out, vs, ls):
    from contextlib import ExitStack

    with ExitStack() as ctx:
        wpool = ctx.enter_context(tc.tile_pool(name="wpool", bufs=1))
        apool = ctx.enter_context(tc.tile_pool(name="apool", bufs=1))
        opool = ctx.enter_context(tc.tile_pool(name="opool", bufs=3))
        spool = ctx.enter_context(tc.tile_pool(name="spool", bufs=2))
        pp = ctx.enter_context(tc.tile_pool(name="pp", bufs=6, space="PSUM"))
        pd = ctx.enter_context(tc.tile_pool(name="pd", bufs=2, space="PSUM"))

        # PE warm-up: ~10 dependency-free matmuls on junk SBUF so the HAM
        # clock gate reaches 8/8 while the first input DMAs are in flight.
        ONES = wpool.tile([P, 2, 16], F8, tag="ones")
        nc.vector.memset(ONES, 1.0)
        JW = wpool.tile([P, 2, P], F8, tag="jw")
        nc.vector.memset(JW, 1.0)
        JM = wpool.tile([P, 2, NF], F8, tag="jm")
        nc.vector.memset(JM, 0.0)
        for w in range(6):
            wps = pp.tile([P, NF], F32, tag="ps", name="warm")
            nc.tensor.matmul(wps, JW, JM, start=True, stop=True, perf_mode=DR)
        QB = wpool.tile([P, NDI], F32, tag="qb")
        KB = wpool.tile([P, NDI], F32, tag="kb")
        VB2 = wpool.tile([P, D], F32, tag="vb")
        WV = wpool.tile([P, NCI, D], BF, tag="wv")
        WQ = wpool.tile([P, NCI, D], F8, tag="wq")
        WK = None

        for b in range(LB):
            v = vs[b]   # valid key chunks for this batch slot
            L = ls[b]   # exact covered key width (<= v*128, multiple of 16)
            # key-dim psum groups: (offset, width) pieces covering L cols.
            # Balanced widths keep every matmul stream-bound; a (512, small)
            # split leaves the small group at the ~60ns NX dispatch floor.
            if L <= NF:
                kg = [(0, L)]
            else:
                w0 = ((L + 1) // 2 + 15) // 16 * 16
                kg = [(0, w0), (w0, L - w0)]

            # --- stage B: QT[d, s] = 16*Q (fp8 DoubleRow; smallest DMA lead-in)
            XT = apool.tile([P, NCI, SP], F8, tag="xt")
            # DMA issue ops cost ~600ns each regardless of size, so batch two
            # 128-chunk loads per XT issue.  WQ instead loads per di column
            # slice: B's psum group di only reads WQ[:, :, di*128:...], so
            # group 0 needs 1.2MB (XT + slice 0) instead of 2.1MB before it
            # can finish — early DMA bandwidth couldn't deliver the latter in
            # time.  b=0 interleaves both issue queues (ScalarE is free until
            # the first QT activation).
            for cp in range(NCI // 2):
                eng = nc.sync if (b > 0 or cp % 2 == 0) else nc.scalar
                eng.dma_start(out=XT[:, 2 * cp:2 * cp + 2, :], in_=xt[b, cp])
                if b == 0:
                    eng = nc.scalar if cp % 2 == 0 else nc.sync
                    eng.dma_start(out=WQ[:, 2 * cp:2 * cp + 2, :], in_=wq[cp])
            MB = spool.tile([P, NKI], F32, tag="mb")
            nc.sync.dma_start(out=MB, in_=mb[b])
            if b == 0:
                nc.sync.dma_start(out=QB, in_=qb[:, :])
                nc.sync.dma_start(out=KB, in_=kb[:, :])
                nc.sync.dma_start(out=VB2, in_=vb[:, :])
            _mark(nc, f"B{b}")
            QT = apool.tile([P, NDI, S], F8, tag="qt")
            for di in range(NDI):
                ps = [pp.tile([P, NF], F32, tag="ps", name=f"ps{_i}") for _i in range(2)]
                for cp in range(0, NCI, 2):
                    lhsT = WQ[:, cp:cp + 2, di * P:(di + 1) * P]
                    for sh in range(2):
                        nc.tensor.matmul(
                            ps[sh], lhsT,
                            XT[:, cp:cp + 2, PADL + sh * NF: PADL + sh * NF + NF],
                            start=(cp == 0), stop=(cp == NCI - 2), perf_mode=DR)
                for sh in range(2):
                    nc.scalar.activation(
                        QT[:, di, sh * NF:(sh + 1) * NF], ps[sh], AF.Identity,
                        bias=QB[:, di:di + 1], scale=AS / WS)

            # --- stage D: V0 natural [s, d]; bf16 Vb2 for +V, fp8 V8 for PV --
            FVT = apool.tile([P, NCI, S], BF, tag="fvt")
            for cp in range(NCI // 2):
                nc.sync.dma_start(out=FVT[:, 2 * cp:2 * cp + 2, :], in_=fvt[b, cp])
                if b == 0:
                    nc.sync.dma_start(out=WV[:, 2 * cp:2 * cp + 2, :], in_=wv[cp])
            _mark(nc, f"D{b}")
            V = apool.tile([P, NSI, D], BF, tag="v")
            V8 = apool.tile([P, NSI, D], F8, tag="v8")
            for si in range(NSI):
                ps = [pp.tile([P, NF], F32, tag="ps", name=f"ps{_i}") for _i in range(2)]
                for ci in range(NCI):
                    lhsT = FVT[:, ci, si * P:(si + 1) * P]
                    for dh in range(2):
                        nc.tensor.matmul(
                            ps[dh], lhsT, WV[:, ci, dh * NF:(dh + 1) * NF],
                            start=(ci == 0), stop=(ci == NCI - 1))
                for dh in range(2):
                    nc.vector.tensor_add(
                        V[:, si, dh * NF:(dh + 1) * NF], ps[dh],
                        VB2[:, dh * NF:(dh + 1) * NF])
                    if si < v:
                        # ScalarE is idle during stage D; without this the DVE
                        # runs ~2.4us/group against the PE's 1.76us cadence
                        nc.scalar.activation(
                            V8[:, si, dh * NF:(dh + 1) * NF], ps[dh],
                            AF.Copy, bias=0.0, scale=1.0)

            # --- stage C: KT[d, s] = 16*K (width-5 stencil, only v key chunks)
            if WK is None:
                WK = []
                for j in range(5):
                    t = wpool.tile([P, NCI, D], F8, tag=f"wk{j}")
                    nc.sync.dma_start(out=t, in_=wk[j])
                    WK.append(t)
            _mark(nc, f"C{b}")
            KT = apool.tile([P, NDI, S], F8, tag="kt")
            if L < v * P:
                # stage E reads whole 128-col chunks; zero the K columns the
                # stencil no longer computes (they are all masked anyway)
                nc.vector.memset(KT[:, :, L:v * P], 0.0)
            for di in range(NDI):
                ps = [pp.tile([P, NF], F32, tag="ps", name=f"ps{_i}")
                      for _i in range(len(kg))]
                step, nsteps = 0, 5 * (NCI // 2)
                for j in range(5):
                    for cp in range(0, NCI, 2):
                        lhsT = WK[j][:, cp:cp + 2, di * P:(di + 1) * P]
                        for g, (off, w) in enumerate(kg):
                            nc.tensor.matmul(
                                ps[g][:, :w], lhsT,
                                XT[:, cp:cp + 2, j + off: j + off + w],
                                start=(step == 0), stop=(step == nsteps - 1),
                                perf_mode=DR)
                        step += 1
                for g, (off, w) in enumerate(kg):
                    nc.scalar.activation(
                        KT[:, di, off:off + w], ps[g][:, :w], AF.Identity,
                        bias=KB[:, di:di + 1], scale=AS / WS)

            # --- stage E: ET16[k, q] = 16*exp(scoresT/32 + mask) -------------
            _mark(nc, f"E{b}")
            ET = apool.tile([P, NKI, S], F8, tag="et")
            for ki in range(v):
                ps = [pp.tile([P, NF], F32, tag="ps", name=f"ps{_i}") for _i in range(2)]
                for dp in range(0, NDI, 2):
                    lhsT = KT[:, dp:dp + 2, ki * P:(ki + 1) * P]
                    for qh in range(2):
                        nc.tensor.matmul(
                            ps[qh], lhsT, QT[:, dp:dp + 2, qh * NF:(qh + 1) * NF],
                            start=(dp == 0), stop=(dp == NDI - 2), perf_mode=DR)
                for qh in range(2):
                    nc.scalar.activation(
                        ET[:, ki, qh * NF:(qh + 1) * NF], ps[qh], AF.Exp,
                        bias=MB[:, ki:ki + 1], scale=SCALE / (AS * AS))

            # --- stage F: out = (ET16^T @ V8) / den + Vb2 --------------------
            _mark(nc, f"F{b}")
            for qi in range(NQI):
                pso = [pp.tile([P, NF], F32, tag="ps", name=f"pso{_i}") for _i in range(2)]
                psd = pd.tile([P, 1], F32, tag="den")
                for kp in range(0, v - (v % 2), 2):
                    lhsT = ET[:, kp:kp + 2, qi * P:(qi + 1) * P]
                    st, sp_ = (kp == 0), (kp + 2 >= v)
                    for dh in range(2):
                        nc.tensor.matmul(
                            pso[dh], lhsT, V8[:, kp:kp + 2, dh * NF:(dh + 1) * NF],
                            start=st, stop=sp_, perf_mode=DR)
                    nc.tensor.matmul(psd, lhsT, ONES[:, :, 0:1],
                                     start=st, stop=sp_, perf_mode=DR)
                if v % 2:
                    ki = v - 1
                    lhsT = ET[:, ki, qi * P:(qi + 1) * P]
                    st = (v == 1)
                    for dh in range(2):
                        nc.tensor.matmul(
                            pso[dh], lhsT, V8[:, ki, dh * NF:(dh + 1) * NF],
                            start=st, stop=True)
                    nc.tensor.matmul(psd, lhsT, ONES[:, 0, 0:1],
                                     start=st, stop=True)
                REC = spool.tile([P, 1], F32, tag="rec")
                nc.vector.reciprocal(REC, psd)
                for dh in range(2):
                    OB = opool.tile([P, NF], BF, tag="obf", name=f"ob{dh}")
                    last = (b == LB - 1 and qi == NQI - 1)
                    if qi >= 2 and not (last and dh == 1):
                        # scalar does the x(1/den) move to bf16; the DVE add is
                        # then all-16-bit (2x rate) -> DVE drops from 1.67us to
                        # ~0.9us per group, ending den-matmul stalls on PSUM
                        OT = opool.tile([P, NF], BF, tag="otb", name=f"otb{dh}")
                        nc.scalar.activation(OT, pso[dh], AF.Copy,
                                             bias=0.0, scale=REC)
                        nc.vector.tensor_add(
                            OB, OT, V[:, qi, dh * NF:(dh + 1) * NF])
                    else:
                        nc.vector.scalar_tensor_tensor(
                            OB, pso[dh], REC, V[:, qi, dh * NF:(dh + 1) * NF],
                            mybir.AluOpType.mult, mybir.AluOpType.add)
                    eng = nc.sync if dh == 0 else nc.scalar
                    eng.dma_start(
                        out=out[b, qi * P:(qi + 1) * P, dh * NF:(dh + 1) * NF],
                        in_=OB)


def _prep_host(feaQK, feaV, seqlengths, cn3_w, cn3_b, cn5_w, cn5_b,
               k_w, k_b, q_w, q_b, v_w, v_b):
    """Compose weights, assign batches to cores, lay out per-core inputs."""
    f32 = np.float32
    bf16 = ml_dtypes.bfloat16
    f8 = ml_dtypes.float8_e4m3
    feaQK = np.asarray(feaQK, f32)
    feaV = np.asarray(feaV, f32)
    seqlengths = np.asarray(seqlengths).astype(np.int64)

    W1 = np.asarray(k_w, f32)[:, :C]
    W2 = np.asarray(k_w, f32)[:, C:2 * C]
    W3 = np.asarray(k_w, f32)[:, 2 * C:]

    wk = np.zeros((5, C, D), f32)  # [tap j (= shift+2), c, d]
    for t in range(3):
        wk[t + 1] += (W2 @ np.asarray(cn3_w, f32)[:, :, t]).T
    for t in range(5):
        wk[t] += (W3 @ np.asarray(cn5_w, f32)[:, :, t]).T
    wk[2] += W1.T
    kb_eff = (np.asarray(k_b, f32) + W2 @ np.asarray(cn3_b, f32)
              + W3 @ np.asarray(cn5_b, f32))

    wq = np.ascontiguousarray(np.asarray(q_w, f32).T)
    wv = np.ascontiguousarray(np.asarray(v_w, f32).T)

    qb_pd = np.ascontiguousarray((np.asarray(q_b, f32) * AS).reshape(NDI, P).T)
    kb_pd = np.ascontiguousarray((kb_eff * AS).reshape(NDI, P).T)
    vb2_rep = np.ascontiguousarray(
        np.broadcast_to(2.0 * np.asarray(v_b, f32), (P, D)))

    key_valid = np.arange(S)[None, :] < seqlengths[:, None]
    mask = np.where(key_valid, LOG_AS, MASK_NEG).astype(f32)  # [B, S]

    # Pair longest with shortest so the compile-time per-slot chunk counts
    # (max over cores) stay near the per-core optimum.
    vchunks = np.clip(np.ceil(seqlengths / P).astype(int), 1, NKI)
    order = np.argsort(-seqlengths, kind="stable")
    batch_of = np.zeros((NCORES, LB), int)
    for i in range(NCORES):
        batch_of[i, 0] = order[B - 1 - i]
        batch_of[i, 1] = order[i]
    vs = (int(vchunks[batch_of[:, 0]].max()),
          int(vchunks[batch_of[:, 1]].max()))
    ls = tuple(min(S, (int(seqlengths[batch_of[:, sl]].max()) + 15) // 16 * 16)
               for sl in range(LB))

    def pairs(a):  # [C, X] -> [NCI//2, P, 2, X] so one DMA covers 2 chunks
        return np.ascontiguousarray(
            a.reshape(NCI // 2, 2, P, -1).transpose(0, 2, 1, 3))

    wq_8 = pairs(np.clip(wq * WS, -240, 240).astype(f8))
    wk_8 = np.ascontiguousarray(
        np.clip(wk * WS, -240, 240).astype(f8).reshape(5, NCI, P, D)
        .transpose(0, 2, 1, 3))
    wv_b = pairs(wv.astype(bf16))

    in_maps = []
    for core in range(NCORES):
        bs = batch_of[core]
        xts = np.zeros((LB, C, SP), f8)
        xts[:, :, PADL:PADL + S] = np.clip(
            feaQK[bs].transpose(0, 2, 1), -240, 240).astype(f8)
        xts = np.ascontiguousarray(
            xts.reshape(LB, NCI // 2, 2, P, SP).transpose(0, 1, 3, 2, 4))
        fvts = np.ascontiguousarray(
            feaV[bs].transpose(0, 2, 1).astype(bf16)
            .reshape(LB, NCI // 2, 2, P, S).transpose(0, 1, 3, 2, 4))
        mbs = np.ascontiguousarray(
            mask[bs].reshape(LB, NKI, P).transpose(0, 2, 1))
        in_maps.append({
            "xt": xts, "fvt": fvts,
            "wq": wq_8, "wk": wk_8, "wv": wv_b,
            "qb": qb_pd, "kb": kb_pd, "vb": vb2_rep, "mb": mbs,
        })
    return in_maps, batch_of, vs, ls


def kernel(**inputs):
    from concourse.bass_utils import run_bass_kernel_spmd

    in_maps, batch_of, vs, ls = _prep_host(**inputs)
    if _CACHE.get("vs") != (vs, ls):
        _CACHE["nc"] = _build_program(vs, ls)
        _CACHE["vs"] = (vs, ls)
    nc = _CACHE["nc"]
    res = run_bass_kernel_spmd(nc, in_maps, core_ids=list(range(NCORES)),
                               trace=TRACE)
    _CACHE["last_result"] = res
    full = np.zeros((B, S, D), np.float32)
    for core in range(NCORES):
        full[batch_of[core]] = res.results[core]["out"].astype(np.float32)
    return full


# revision 27
# speedup vs baseline: 1.1916x; 1.1798x over previous
"""Contextual attention kernel for Trainium2 (8 NeuronCores, data-parallel over batch).

Math (per batch b):
    Q = feaQK @ q_w.T + q_b
    k3 = conv1d(feaQK.T, cn3_w, SAME) + b3 ; k5 = conv1d(..., cn5_w) + b5
    K = [feaQK, k3, k5] @ k_w.T + k_b
    V = feaV @ v_w.T + v_b
    S = (Q @ K.T) / sqrt(D); mask keys >= seqlen with -inf
    out = softmax(S) @ V + V

Kernel strategy:
  * The convs + concat + K-projection collapse into a single width-5 stencil:
        K[s] = sum_{d=-2..2} feaQK[s+d] @ Wk[d] + kb_eff
    with Wk composed on the host (15 matmul-units of work -> 9).
  * All activations live on-chip in transposed layout ([feature, seq]) so no
    on-device transposes are needed anywhere:
        QT/KT from xT (host-transposed feaQK, zero-padded cols)
        scoresT[k,q] = KT chunks (stationary) x QT  (PSUM fp32)
        ET16 = 16*exp(scoresT/32 + mask)  (mask folded into exp bias)
        V0 (natural [s,d]) from host-transposed feaV as the stationary operand
        outU[q,d] = ET16 chunks (stationary) x V8; den[q] = ET16 x ones
        out = outU / den + (V0 + 2*vb)        [atten@vb == vb since sum(atten)=1]
  * fp8e4 DoubleRow matmuls (2 contraction blocks / instruction, ~1.5-1.8x bf16)
    for the Q-proj, K-stencil, scores and PV stages. Weights are scaled x256 on
    the host so they sit in fp8's normal range; Q/K are stored x16; ET x16.
    The V-projection stays bf16: out ~= V, so V's accuracy dominates the
    output and fp8 would blow the error budget. The PV matmul uses a separate
    fp8 copy (V8) of the unbiased projection.
  * Keys beyond seqlength are dead: K/scores/PV work only covers the first
    ceil(seqlen/128) key chunks per batch slot. Batches are paired
    longest-with-shortest across cores so the compile-time per-slot chunk
    counts (max over cores) stay small; sub-chunk masking still goes through
    the exp bias, so over-covering is always correct.
  * 16 batches -> 2 per core, full weights on every core. Output DMA in bf16.
"""

import numpy as np
import ml_dtypes

from concourse import bacc
import concourse.tile as tile
from concourse import mybir

B, S, C, D = 16, 1024, 1024, 1024
P = 128
NCI, NDI, NKI, NQI, NSI = C // P, D // P, S // P, S // P, S // P
NF = 512  # matmul free dim (one PSUM bank of fp32)
PADL = 2   # left zero pad for the width-5 stencil
SP = 1040  # padded seq width; multiple of 16 so fp8 DoubleRow pair-stride is legal
LB = 2  # local batches per core
NCORES = 8
MASK_NEG = -60000.0
SCALE = 1.0 / 32.0  # 1/sqrt(D)
WS = 256.0   # host weight scale into fp8 normal range
AS = 16.0    # on-chip activation scale for QT/KT/ET
LOG_AS = float(np.log(AS))

BF = mybir.dt.bfloat16
F32 = mybir.dt.float32
F8 = mybir.dt.float8e4
AF = mybir.ActivationFunctionType
DR = mybir.MatmulPerfMode.DoubleRow

TRACE = False  # set by test harness to collect HW profile
_CACHE = {}
MARKS = []  # (label, first-instruction-name) per stage, for trace attribution


def _build_program(vs, ls):
    nc = bacc.Bacc("TRN2", dynamic_dma_scratch_size=256)

    xt = nc.dram_tensor("xt", [LB, NCI // 2, P, 2, SP], F8, kind="ExternalInput")
    fvt = nc.dram_tensor("fvt", [LB, NCI // 2, P, 2, S], BF, kind="ExternalInput")
    wq = nc.dram_tensor("wq", [NCI // 2, P, 2, D], F8, kind="ExternalInput")
    wk = nc.dram_tensor("wk", [5, P, NCI, D], F8, kind="ExternalInput")
    wv = nc.dram_tensor("wv", [NCI // 2, P, 2, D], BF, kind="ExternalInput")
    qb = nc.dram_tensor("qb", [P, NDI], F32, kind="ExternalInput")
    kb = nc.dram_tensor("kb", [P, NDI], F32, kind="ExternalInput")
    vb = nc.dram_tensor("vb", [P, D], F32, kind="ExternalInput")
    mb = nc.dram_tensor("mb", [LB, P, NKI], F32, kind="ExternalInput")
    out = nc.dram_tensor("out", [LB, S, D], BF, kind="ExternalOutput")

    with tile.TileContext(nc) as tc:
        _emit(nc, tc, xt, fvt, wq, wk, wv, qb, kb, vb, mb, out, vs, ls)
    nc.finalize()
    return nc


def _mark(nc, label):
    mx = 0
    for k in nc._state.inst_map:
        if k.startswith("I-"):
            try:
                mx = max(mx, int(k[2:].split("_")[0]))
            except ValueError:
                pass
    MARKS.append((label, mx))


def _emit(nc, tc, xt, fvt, wq, wk, wv, qb, kb, vb, mb, out, vs, ls):
    from contextlib import ExitStack

    with ExitStack() as ctx:
        wpool = ctx.enter_context(tc.tile_pool(name="wpool", bufs=1))
        apool = ctx.enter_context(tc.tile_pool(name="apool", bufs=1))
        opool = ctx.enter_context(tc.tile_pool(name="opool", bufs=3))
        spool = ctx.enter_context(tc.tile_pool(name="spool", bufs=2))
        pp = ctx.enter_context(tc.tile_pool(name="pp", bufs=6, space="PSUM"))
        pd = ctx.enter_context(tc.tile_pool(name="pd", bufs=2, space="PSUM"))

        # PE warm-up: ~10 dependency-free matmuls on junk SBUF so the HAM
        # clock gate reaches 8/8 while the first input DMAs are in flight.
        ONES = wpool.tile([P, 2, 16], F8, tag="ones")
        nc.vector.memset(ONES, 1.0)
        JW = wpool.tile([P, 2, P], F8, tag="jw")
        nc.vector.memset(JW, 1.0)
        JM = wpool.tile([P, 2, NF], F8, tag="jm")
        nc.vector.memset(JM, 0.0)
        for w in range(6):
            wps = pp.tile([P, NF], F32, tag="ps", name="warm")
            nc.tensor.matmul(wps, JW, JM, start=True, stop=True, perf_mode=DR)
        QB = wpool.tile([P, NDI], F32, tag="qb")
        KB = wpool.tile([P, NDI], F32, tag="kb")
        VB2 = wpool.tile([P, D], F32, tag="vb")
        WV = wpool.tile([P, NCI, D], BF, tag="wv")
        WQ = wpool.tile([P, NCI, D], F8, tag="wq")
        WK = None

        for b in range(LB):
            v = vs[b]   # valid key chunks for this batch slot
            L = ls[b]   # exact covered key width (<= v*128, multiple of 16)
            # key-dim psum groups: (offset, width) pieces covering L cols.
            # Balanced widths keep every matmul stream-bound; a (512, small)
            # split leaves the small group at the ~60ns NX dispatch floor.
            if L <= NF:
                kg = [(0, L)]
            else:
                w0 = ((L + 1) // 2 + 15) // 16 * 16
                kg = [(0, w0), (w0, L - w0)]

            # --- stage B: QT[d, s] = 16*Q (fp8 DoubleRow; smallest DMA lead-in)
            XT = apool.tile([P, NCI, SP], F8, tag="xt")
            # DMA issue ops cost ~600ns each regardless of size, so batch two
            # 128-chunk loads per issue.  b=0: ScalarE has no work queued until
            # the first QT activation, so use it as a second issue queue, with
            # the first matmul's dependencies (XT01, WQ01) first on each.
            for cp in range(NCI // 2):
                eng = nc.sync if (b > 0 or cp % 2 == 0) else nc.scalar
                eng.dma_start(out=XT[:, 2 * cp:2 * cp + 2, :], in_=xt[b, cp])
                if b == 0:
                    eng = nc.scalar if cp % 2 == 0 else nc.sync
                    eng.dma_start(out=WQ[:, 2 * cp:2 * cp + 2, :], in_=wq[cp])
            MB = spool.tile([P, NKI], F32, tag="mb")
            nc.sync.dma_start(out=MB, in_=mb[b])
            if b == 0:
                nc.sync.dma_start(out=QB, in_=qb[:, :])
                nc.sync.dma_start(out=KB, in_=kb[:, :])
                nc.sync.dma_start(out=VB2, in_=vb[:, :])
            _mark(nc, f"B{b}")
            QT = apool.tile([P, NDI, S], F8, tag="qt")
            for di in range(NDI):
                ps = [pp.tile([P, NF], F32, tag="ps", name=f"ps{_i}") for _i in range(2)]
                for cp in range(0, NCI, 2):
                    lhsT = WQ[:, cp:cp + 2, di * P:(di + 1) * P]
                    for sh in range(2):
                        nc.tensor.matmul(
                            ps[sh], lhsT,
                            XT[:, cp:cp + 2, PADL + sh * NF: PADL + sh * NF + NF],
                            start=(cp == 0), stop=(cp == NCI - 2), perf_mode=DR)
                for sh in range(2):
                    nc.scalar.activation(
                        QT[:, di, sh * NF:(sh + 1) * NF], ps[sh], AF.Identity,
                        bias=QB[:, di:di + 1], scale=AS / WS)

            # --- stage D: V0 natural [s, d]; bf16 Vb2 for +V, fp8 V8 for PV --
            FVT = apool.tile([P, NCI, S], BF, tag="fvt")
            for cp in range(NCI // 2):
                nc.sync.dma_start(out=FVT[:, 2 * cp:2 * cp + 2, :], in_=fvt[b, cp])
                if b == 0:
                    nc.sync.dma_start(out=WV[:, 2 * cp:2 * cp + 2, :], in_=wv[cp])
            _mark(nc, f"D{b}")
            V = apool.tile([P, NSI, D], BF, tag="v")
            V8 = apool.tile([P, NSI, D], F8, tag="v8")
            for si in range(NSI):
                ps = [pp.tile([P, NF], F32, tag="ps", name=f"ps{_i}") for _i in range(2)]
                for ci in range(NCI):
                    lhsT = FVT[:, ci, si * P:(si + 1) * P]
                    for dh in range(2):
                        nc.tensor.matmul(
                            ps[dh], lhsT, WV[:, ci, dh * NF:(dh + 1) * NF],
                            start=(ci == 0), stop=(ci == NCI - 1))
                for dh in range(2):
                    nc.vector.tensor_add(
                        V[:, si, dh * NF:(dh + 1) * NF], ps[dh],
                        VB2[:, dh * NF:(dh + 1) * NF])
                    if si < v:
                        # ScalarE is idle during stage D; without this the DVE
                        # runs ~2.4us/group against the PE's 1.76us cadence
                        nc.scalar.activation(
                            V8[:, si, dh * NF:(dh + 1) * NF], ps[dh],
                            AF.Copy, bias=0.0, scale=1.0)

            # --- stage C: KT[d, s] = 16*K (width-5 stencil, only v key chunks)
            if WK is None:
                WK = []
                for j in range(5):
                    t = wpool.tile([P, NCI, D], F8, tag=f"wk{j}")
                    nc.sync.dma_start(out=t, in_=wk[j])
                    WK.append(t)
            _mark(nc, f"C{b}")
            KT = apool.tile([P, NDI, S], F8, tag="kt")
            if L < v * P:
                # stage E reads whole 128-col chunks; zero the K columns the
                # stencil no longer computes (they are all masked anyway)
                nc.vector.memset(KT[:, :, L:v * P], 0.0)
            for di in range(NDI):
                ps = [pp.tile([P, NF], F32, tag="ps", name=f"ps{_i}")
                      for _i in range(len(kg))]
                step, nsteps = 0, 5 * (NCI // 2)
                for j in range(5):
                    for cp in range(0, NCI, 2):
                        lhsT = WK[j][:, cp:cp + 2, di * P:(di + 1) * P]
                        for g, (off, w) in enumerate(kg):
                            nc.tensor.matmul(
                                ps[g][:, :w], lhsT,
                                XT[:, cp:cp + 2, j + off: j + off + w],
                                start=(step == 0), stop=(step == nsteps - 1),
                                perf_mode=DR)
                        step += 1
                for g, (off, w) in enumerate(kg):
                    nc.scalar.activation(
                        KT[:, di, off:off + w], ps[g][:, :w], AF.Identity,
                        bias=KB[:, di:di + 1], scale=AS / WS)

            # --- stage E: ET16[k, q] = 16*exp(scoresT/32 + mask) -------------
            _mark(nc, f"E{b}")
            ET = apool.tile([P, NKI, S], F8, tag="et")
            for ki in range(v):
                ps = [pp.tile([P, NF], F32, tag="ps", name=f"ps{_i}") for _i in range(2)]
                for dp in range(0, NDI, 2):
                    lhsT = KT[:, dp:dp + 2, ki * P:(ki + 1) * P]
                    for qh in range(2):
                        nc.tensor.matmul(
                            ps[qh], lhsT, QT[:, dp:dp + 2, qh * NF:(qh + 1) * NF],
                            start=(dp == 0), stop=(dp == NDI - 2), perf_mode=DR)
                for qh in range(2):
                    nc.scalar.activation(
                        ET[:, ki, qh * NF:(qh + 1) * NF], ps[qh], AF.Exp,
                        bias=MB[:, ki:ki + 1], scale=SCALE / (AS * AS))

            # --- stage F: out = (ET16^T @ V8) / den + Vb2 --------------------
            _mark(nc, f"F{b}")
            for qi in range(NQI):
                pso = [pp.tile([P, NF], F32, tag="ps", name=f"pso{_i}") for _i in range(2)]
                psd = pd.tile([P, 1], F32, tag="den")
                for kp in range(0, v - (v % 2), 2):
                    lhsT = ET[:, kp:kp + 2, qi * P:(qi + 1) * P]
                    st, sp_ = (kp == 0), (kp + 2 >= v)
                    for dh in range(2):
                        nc.tensor.matmul(
                            pso[dh], lhsT, V8[:, kp:kp + 2, dh * NF:(dh + 1) * NF],
                            start=st, stop=sp_, perf_mode=DR)
                    nc.tensor.matmul(psd, lhsT, ONES[:, :, 0:1],
                                     start=st, stop=sp_, perf_mode=DR)
                if v % 2:
                    ki = v - 1
                    lhsT = ET[:, ki, qi * P:(qi + 1) * P]
                    st = (v == 1)
                    for dh in range(2):
                        nc.tensor.matmul(
                            pso[dh], lhsT, V8[:, ki, dh * NF:(dh + 1) * NF],
                            start=st, stop=True)
                    nc.tensor.matmul(psd, lhsT, ONES[:, 0, 0:1],
                                     start=st, stop=True)
                REC = spool.tile([P, 1], F32, tag="rec")
                nc.vector.reciprocal(REC, psd)
                for dh in range(2):
                    OB = opool.tile([P, NF], BF, tag="obf", name=f"ob{dh}")
                    last = (b == LB - 1 and qi == NQI - 1)
                    if qi >= 2 and not (last and dh == 1):
                        # scalar does the x(1/den) move to bf16; the DVE add is
                        # then all-16-bit (2x rate) -> DVE drops from 1.67us to
                        # ~0.9us per group, ending den-matmul stalls on PSUM
                        OT = opool.tile([P, NF], BF, tag="otb", name=f"otb{dh}")
                        nc.scalar.activation(OT, pso[dh], AF.Copy,
                                             bias=0.0, scale=REC)
                        nc.vector.tensor_add(
                            OB, OT, V[:, qi, dh * NF:(dh + 1) * NF])
                    else:
                        nc.vector.scalar_tensor_tensor(
                            OB, pso[dh], REC, V[:, qi, dh * NF:(dh + 1) * NF],
                            mybir.AluOpType.mult, mybir.AluOpType.add)
                    eng = nc.sync if dh == 0 else nc.scalar
                    eng.dma_start(
                        out=out[b, qi * P:(qi + 1) * P, dh * NF:(dh + 1) * NF],
                        in_=OB)


def _prep_host(feaQK, feaV, seqlengths, cn3_w, cn3_b, cn5_w, cn5_b,
               k_w, k_b, q_w, q_b, v_w, v_b):
    """Compose weights, assign batches to cores, lay out per-core inputs."""
    f32 = np.float32
    bf16 = ml_dtypes.bfloat16
    f8 = ml_dtypes.float8_e4m3
    feaQK = np.asarray(feaQK, f32)
    feaV = np.asarray(feaV, f32)
    seqlengths = np.asarray(seqlengths).astype(np.int64)

    W1 = np.asarray(k_w, f32)[:, :C]
    W2 = np.asarray(k_w, f32)[:, C:2 * C]
    W3 = np.asarray(k_w, f32)[:, 2 * C:]

    wk = np.zeros((5, C, D), f32)  # [tap j (= shift+2), c, d]
    for t in range(3):
        wk[t + 1] += (W2 @ np.asarray(cn3_w, f32)[:, :, t]).T
    for t in range(5):
        wk[t] += (W3 @ np.asarray(cn5_w, f32)[:, :, t]).T
    wk[2] += W1.T
    kb_eff = (np.asarray(k_b, f32) + W2 @ np.asarray(cn3_b, f32)
              + W3 @ np.asarray(cn5_b, f32))

    wq = np.ascontiguousarray(np.asarray(q_w, f32).T)
    wv = np.ascontiguousarray(np.asarray(v_w, f32).T)

    qb_pd = np.ascontiguousarray((np.asarray(q_b, f32) * AS).reshape(NDI, P).T)
    kb_pd = np.ascontiguousarray((kb_eff * AS).reshape(NDI, P).T)
    vb2_rep = np.ascontiguousarray(
        np.broadcast_to(2.0 * np.asarray(v_b, f32), (P, D)))

    key_valid = np.arange(S)[None, :] < seqlengths[:, None]
    mask = np.where(key_valid, LOG_AS, MASK_NEG).astype(f32)  # [B, S]

    # Pair longest with shortest so the compile-time per-slot chunk counts
    # (max over cores) stay near the per-core optimum.
    vchunks = np.clip(np.ceil(seqlengths / P).astype(int), 1, NKI)
    order = np.argsort(-seqlengths, kind="stable")
    batch_of = np.zeros((NCORES, LB), int)
    for i in range(NCORES):
        batch_of[i, 0] = order[B - 1 - i]
        batch_of[i, 1] = order[i]
    vs = (int(vchunks[batch_of[:, 0]].max()),
          int(vchunks[batch_of[:, 1]].max()))
    ls = tuple(min(S, (int(seqlengths[batch_of[:, sl]].max()) + 15) // 16 * 16)
               for sl in range(LB))

    def pairs(a):  # [C, X] -> [NCI//2, P, 2, X] so one DMA covers 2 chunks
        return np.ascontiguousarray(
            a.reshape(NCI // 2, 2, P, -1).transpose(0, 2, 1, 3))

    wq_8 = pairs(np.clip(wq * WS, -240, 240).astype(f8))
    wk_8 = np.ascontiguousarray(
        np.clip(wk * WS, -240, 240).astype(f8).reshape(5, NCI, P, D)
        .transpose(0, 2, 1, 3))
    wv_b = pairs(wv.astype(bf16))

    in_maps = []
    for core in range(NCORES):
        bs = batch_of[core]
        xts = np.zeros((LB, C, SP), f8)
        xts[:, :, PADL:PADL + S] = np.clip(
            feaQK[bs].transpose(0, 2, 1), -240, 240).astype(f8)
        xts = np.ascontiguousarray(
            xts.reshape(LB, NCI // 2, 2, P, SP).transpose(0, 1, 3, 2, 4))
        fvts = np.ascontiguousarray(
            feaV[bs].transpose(0, 2, 1).astype(bf16)
            .reshape(LB, NCI // 2, 2, P, S).transpose(0, 1, 3, 2, 4))
        mbs = np.ascontiguousarray(
            mask[bs].reshape(LB, NKI, P).transpose(0, 2, 1))
        in_maps.append({
            "xt": xts, "fvt": fvts,
            "wq": wq_8, "wk": wk_8, "wv": wv_b,
            "qb": qb_pd, "kb": kb_pd, "vb": vb2_rep, "mb": mbs,
        })
    return in_maps, batch_of, vs, ls


def kernel(**inputs):
    from concourse.bass_utils import run_bass_kernel_spmd

    in_maps, batch_of, vs, ls = _prep_host(**inputs)
    if _CACHE.get("vs") != (vs, ls):
        _CACHE["nc"] = _build_program(vs, ls)
        _CACHE["vs"] = (vs, ls)
    nc = _CACHE["nc"]
    res = run_bass_kernel_spmd(nc, in_maps, core_ids=list(range(NCORES)),
                               trace=TRACE)
    _CACHE["last_result"] = res
    full = np.zeros((B, S, D), np.float32)
    for core in range(NCORES):
        full[batch_of[core]] = res.results[core]["out"].astype(np.float32)
    return full
